# revision 36
# baseline (speedup 1.0000x reference)
"""Trainium2 Bass kernel for nn_DentalAnatomyLoss.

Computes, for segmentation [B=2, C=32, D=64, H=128, W=128] fp32:
  - crown/root ratio loss (per (b,c) sums over d<32 / d>=32)
  - 3D total-variation loss (mean |diff| along w, h, d)
  - returns stack([crown_root, smoothness, total_anatomy]) fp32 [3]

Strategy: pure data-parallel over the 64 (b,c) slices, 8 per NeuronCore.
Each core reduces its 32 MiB shard to a [128, 144] fp32 partial tensor;
the host combines partials into the 3 scalars.

Layout: partition p = 2*d + s where s = h//64, free f = (r, w) with
r = h % 64.  The DMA loads each slice as one [128, 8192] transfer whose
per-partition source is a contiguous 32 KiB block (d-plane half-row),
and casts fp32 -> bf16 in the SDMA datapath (SWDGE), so no engine pass
is spent on the cast and HBM traffic is the fp32 read only.

Per-core engine split (memory regime, ~94 us HBM roofline/core):
  - TensorE: d-diffs (gz) via a block-bidiagonal matmul (the d axis sits
    on partitions); columns 126/127 of the same stationary carry
    crown/ones indicator vectors so the crown/total sums ride along in
    otherwise-zero psum rows.  Two more tiny stationaries compute the
    h=63|64 boundary row diffs (dual accumulated matmuls) and the
    d=0 / d=63 plane sums (for the gz telescoping term).
  - ScalarE: relu+accum drains of all psum tiles.
  - VectorE: h-diffs (gy) as aligned bf16 subtract (2x) + fused
    relu-sum (4x); w-diffs (gx) as one fused max+accum (1x, the
    shift-by-one AP cannot reach a packed mode); tiny row/col sums for
    the telescoping identities sum|a-b| = 2*sum(max/relu) - signed sums.
"""

import os

import numpy as np

B, C, D, H, W = 2, 32, 64, 128, 128
NCORES = 8
JPC = (B * C) // NCORES  # (b,c) slices per core
CROWN_ROOT_W = 2.0
SMOOTH_W = 1.5
EXPECTED_RATIO = 1.2

# accumulator column layout in the [128, ACC_COLS] partial tensor
ACC_COLS = 144
NT = 6       # psum drain tiles per slice: 5 x 1536 + remainder
GZ0 = 0      # 48: per gz-psum-tile relu sums (6 tiles x 8 slices);
             #     rows 0..125 = relu(dz), row 126 = crown, row 127 = total
GXP = 48     # 12: gx-via-PE relu drains (6 tiles x 2 slices)
GXB = 60     # 8: sum of relu/max over bogus w-wrap pairs, per slice
GYR = 68     # 8: sum(relu(dy_internal)) per slice
GYS0 = 76    # 8: per-partition rowsum r=0 per slice
GYS1 = 84    # 8: per-partition rowsum r=63 per slice
GX = 92      # 8: sum(max(x_w, x_{w+1})) per slice (STT / DMA-max routes)
GXC0 = 100   # 8: per-partition colsum w=0 per slice
GXC1 = 108   # 8: per-partition colsum w=127 per slice
PLC = 116    # 1: rows 0/1 = plane d=0 / d=63 sums (global, all slices)
PBR = 120    # 8: sum(relu(boundary dy)) per slice
# (sum(boundary dy) is recovered on host from GYS0/GYS1)
# 128:144 unused (zeroed)

# gx routing per slice: VectorE fused max (STT, 1x), DMA shift-copy +
# aligned TT-max (2x) + TS-sum (4x) (DCOPY), or PE dual-matmul diffs (PE).
# (SDMA accum_op=max is rejected by the BIR verifier - "DMACopy does not
# support max with Copy mode" - so the copy is plain and VectorE maxes.)
GX_STT = ()
GX_DCOPY = (0, 1, 2, 3, 4, 5)
GXPE_SLICES = (6, 7)
GY_DMAX = ()  # SDMA max-accumulate unsupported; gy runs on VectorE
GYM = 132

_PROG_CACHE: dict = {}
last_exec_time_ns = None  # set by kernel() when tracing is enabled


def _build_program(jpc=JPC, d=D, h=H, w=W, repeat=1, small_input=False):
    """Build the (single) SPMD Bass program run identically on all cores.

    repeat>1 wraps the whole compute in a hardware For_i loop (identical
    result, used only for wall-clock timing of the kernel body).
    small_input shrinks the dram input to one slice (re-read jpc times)
    so timing runs ship 8x less data through the tunnel.
    """
    from contextlib import ExitStack

    import concourse.tile as tile
    from concourse import bacc, mybir

    f32 = mybir.dt.float32
    bf16 = mybir.dt.bfloat16
    AO = mybir.AluOpType
    AF = mybir.ActivationFunctionType

    assert (d, h, w) == (64, 128, 128), "layout is hardcoded for 64x128x128"
    hh = h // 2          # rows per partition-half (64)
    fsz = hh * w         # free size per partition (8192)
    nblk = fsz // 512    # 512-blocks per slice (16)

    nc = bacc.Bacc(
        "TRN2",
        target_bir_lowering=False,
        debug=False,
        enable_asserts=False,
        num_devices=NCORES,
    )
    jdram = 1 if small_input else jpc
    seg = nc.dram_tensor("seg", [jdram, d, h, w], f32, kind="ExternalInput").ap()
    mats = nc.dram_tensor("mats", [128, 5 * 128 + 2], bf16, kind="ExternalInput").ap()
    out = nc.dram_tensor("partials", [128, ACC_COLS], f32, kind="ExternalOutput").ap()

    with tile.TileContext(nc) as tc, ExitStack() as ctx:
        singles = ctx.enter_context(tc.tile_pool(name="singles", bufs=1))
        xbp = ctx.enter_context(tc.tile_pool(name="xb", bufs=3))
        dyp = ctx.enter_context(tc.tile_pool(name="dy", bufs=2))
        dxp = ctx.enter_context(tc.tile_pool(name="dx", bufs=2))
        tinyp = ctx.enter_context(tc.tile_pool(name="tiny", bufs=2))
        dummyp = ctx.enter_context(tc.tile_pool(name="dummy", bufs=4))
        pszp = ctx.enter_context(tc.tile_pool(name="psz", bufs=2, space="PSUM"))
        psbp = ctx.enter_context(tc.tile_pool(name="psb", bufs=1, space="PSUM"))
        pslp = ctx.enter_context(tc.tile_pool(name="psl", bufs=1, space="PSUM"))

        mats_sb = singles.tile([128, 5 * 128 + 2], bf16)
        nc.sync.dma_start(out=mats_sb, in_=mats)
        Bz = mats_sb[:, 0:128]
        A1 = mats_sb[:, 128:256]
        A2 = mats_sb[:, 256:384]
        PL = mats_sb[:, 384:386]
        Ipos = mats_sb[:, 386:514]
        Ineg = mats_sb[:, 514:642]

        acc = singles.tile([128, ACC_COLS], f32)
        nc.vector.memset(acc, 0.0)

        # plane d=0 / d=63 sums accumulate across ALL slices (their term
        # only feeds the global gz sum), drained once at the end.
        pl = pslp.tile([2, 512], f32)

        def drain(ps_ap, func, col_ap, fd):
            np_ = ps_ap.shape[0]
            dmy = dummyp.tile([128, 1], bf16)
            nc.scalar.activation(
                out=dmy[0:np_, :].broadcast_to((np_, fd)),
                in_=ps_ap,
                func=func,
                accum_out=col_ap,
            )

        def slice_body(j):
            src = seg[0 if small_input else j]
            xb = xbp.tile([128, fsz], bf16)
            nc.gpsimd.dma_start(
                out=xb, in_=src.rearrange("d (s r) w -> (d s) (r w)", s=2)
            )
            x3 = xb.rearrange("p (r w) -> p r w", w=w)

            # ---- VectorE / SDMA ----
            # gy internal: dy = x[h+1] - x[h] (aligned 2x) on VectorE; the
            # relu+sum runs on ScalarE, which does not pay the DVE
            # pipe-drain (every DVE op costs ~2x its streaming time on HW).
            dy = dyp.tile([128, fsz - w], bf16)
            nc.vector.tensor_tensor(
                out=dy, in0=xb[:, w:fsz], in1=xb[:, 0 : fsz - w],
                op=AO.subtract,
            )
            dmy = dummyp.tile([128, 1], bf16)
            nc.scalar.activation(
                out=dmy.broadcast_to((128, fsz - w)),
                in_=dy,
                func=AF.Relu,
                accum_out=acc[:, GYR + j : GYR + j + 1],
            )
            # rowsums r=0 / r=63 for the gy telescoping term
            t0 = tinyp.tile([128, w], bf16)
            nc.vector.tensor_scalar(
                out=t0, in0=xb[:, 0:w], scalar1=0.0, scalar2=None,
                op0=AO.add, op1=AO.add,
                accum_out=acc[:, GYS0 + j : GYS0 + j + 1],
            )
            t1 = tinyp.tile([128, w], bf16)
            nc.vector.tensor_scalar(
                out=t1, in0=xb[:, fsz - w : fsz], scalar1=0.0, scalar2=None,
                op0=AO.add, op1=AO.add,
                accum_out=acc[:, GYS1 + j : GYS1 + j + 1],
            )
            # gx, three routes (see module doc):
            if j in GX_STT:
                # fused max+accum over w-adjacent pairs (1x)
                dx = dxp.tile([128, hh, w - 1], bf16)
                nc.vector.scalar_tensor_tensor(
                    out=dx,
                    in0=x3[:, :, 1:],
                    scalar=0.0,
                    in1=x3[:, :, 0 : w - 1],
                    op0=AO.bypass,
                    op1=AO.max,
                    accum_out=acc[:, GX + j : GX + j + 1],
                )
            elif j in GX_DCOPY:
                # DMA makes an aligned copy of the 2B-misaligned shifted
                # view (plain memcpy is alignment-safe); VectorE then runs
                # an aligned TT-max (2x) + fused sum (4x) over the flat
                # free dim.  Bogus w-wrap maxes subtracted on host (GXB).
                mxs = dxp.tile([128, fsz - 1], bf16)
                nc.sync.dma_start(out=mxs, in_=xb[:, 1:fsz])
                nc.vector.tensor_tensor(
                    out=mxs, in0=xb[:, 0 : fsz - 1], in1=mxs, op=AO.max
                )
                nc.vector.tensor_scalar(
                    out=mxs, in0=mxs, scalar1=0.0, scalar2=None,
                    op0=AO.add, op1=AO.add,
                    accum_out=acc[:, GX + j : GX + j + 1],
                )
                bg = tinyp.tile([128, hh - 1, 1], bf16)
                nc.vector.scalar_tensor_tensor(
                    out=bg,
                    in0=x3[:, 1:, 0:1],
                    scalar=0.0,
                    in1=x3[:, : hh - 1, w - 1 : w],
                    op0=AO.bypass,
                    op1=AO.max,
                    accum_out=acc[:, GXB + j : GXB + j + 1],
                )
            else:
                # relu(bogus w-wrap pairs): dx_flat includes pairs
                # (r, w-1)->(r+1, 0); their relu-sum is subtracted on host.
                bg = tinyp.tile([128, hh - 1, 1], bf16)
                nc.vector.tensor_tensor(
                    out=bg, in0=x3[:, 1:, 0:1], in1=x3[:, : hh - 1, w - 1 : w],
                    op=AO.subtract,
                )
                nc.vector.tensor_scalar(
                    out=bg, in0=bg, scalar1=0.0, scalar2=None,
                    op0=AO.max, op1=AO.add,
                    accum_out=acc[:, GXB + j : GXB + j + 1],
                )
            # colsums w=0 / w=127 for the gx telescoping term
            c0 = tinyp.tile([128, hh, 1], bf16)
            nc.vector.tensor_scalar(
                out=c0, in0=x3[:, :, 0:1], scalar1=0.0, scalar2=None,
                op0=AO.add, op1=AO.add,
                accum_out=acc[:, GXC0 + j : GXC0 + j + 1],
            )
            c1 = tinyp.tile([128, hh, 1], bf16)
            nc.vector.tensor_scalar(
                out=c1, in0=x3[:, :, w - 1 : w], scalar1=0.0, scalar2=None,
                op0=AO.add, op1=AO.add,
                accum_out=acc[:, GXC1 + j : GXC1 + j + 1],
            )

            # ---- TensorE + ScalarE ----
            # gz: dz rows q=2d+s = x[d+1] - x[d]; rows 126/127 carry
            # crown/total column sums (>=0, so the relu drain is exact).
            # Ragged psum tiles (5 x 1536 + 512) amortize the per-drain
            # fixed cost on ScalarE.
            for t in range(NT):
                base = t * 1536
                size = min(1536, fsz - base)
                ps = pszp.tile([128, 1536], mybir.dt.float32)
                for blk in range(size // 512):
                    b0 = base + blk * 512
                    nc.tensor.matmul(
                        ps[:, blk * 512 : (blk + 1) * 512],
                        Bz,
                        xb[:, b0 : b0 + 512],
                        start=True,
                        stop=True,
                    )
                col = GZ0 + j * NT + t
                drain(ps[:, 0:size], AF.Relu, acc[:, col : col + 1], size)

            # gx via PE for the offloaded slices: psum = x[f+1] - x[f]
            # over the flat free dim (bogus wrap pairs corrected on host).
            if j in GXPE_SLICES:
                jx = j - GXPE_SLICES[0]
                for t in range(NT):
                    base = t * 1536
                    size = min(1536, fsz - 1 - base)
                    ps = pszp.tile([128, 1536], mybir.dt.float32)
                    nbk = (size + 511) // 512
                    for blk in range(nbk):
                        b0 = base + blk * 512
                        blen = min(512, size - blk * 512)
                        nc.tensor.matmul(
                            ps[:, blk * 512 : blk * 512 + blen],
                            Ipos,
                            xb[:, b0 + 1 : b0 + 1 + blen],
                            start=True,
                            stop=False,
                        )
                    for blk in range(nbk):
                        b0 = base + blk * 512
                        blen = min(512, size - blk * 512)
                        nc.tensor.matmul(
                            ps[:, blk * 512 : blk * 512 + blen],
                            Ineg,
                            xb[:, b0 : b0 + blen],
                            start=False,
                            stop=True,
                        )
                    col = GXP + jx * NT + t
                    drain(ps[:, 0:size], AF.Relu, acc[:, col : col + 1], size)

            # plane d=0 / d=63 sums (gz telescoping), accumulated over
            # blocks AND slices into the long-lived pl psum tile
            for blk in range(nblk):
                nc.tensor.matmul(
                    pl,
                    PL,
                    xb[:, blk * 512 : (blk + 1) * 512],
                    start=(j == 0 and blk == 0),
                    stop=(j == jpc - 1 and blk == nblk - 1),
                )

            # gy boundary rows h=63|64: pb[2d, :] = x[2d+1, 0:w] - x[2d, fsz-w:]
            # (its plain sum is recovered on host from the GYS0/GYS1 rowsums)
            pb = psbp.tile([128, w], mybir.dt.float32)
            nc.tensor.matmul(pb, A1, xb[:, 0:w], start=True, stop=False)
            nc.tensor.matmul(pb, A2, xb[:, fsz - w : fsz], start=False, stop=True)
            # relu+sum drained on VectorE (in-place, psum) to offload ScalarE
            nc.vector.tensor_scalar(
                out=pb, in0=pb, scalar1=0.0, scalar2=None,
                op0=AO.max, op1=AO.add,
                accum_out=acc[:, PBR + j : PBR + j + 1],
            )

        def all_slices():
            for j in range(jpc):
                slice_body(j)
            drain(pl, AF.Relu, acc[0:2, PLC : PLC + 1], 512)

        if repeat == 1:
            all_slices()
        else:
            with tc.For_i(0, repeat, 1):
                all_slices()
        nc.sync.dma_start(out=out, in_=acc)

    nc.compile()
    return nc


def _get_program():
    key = "full"
    if key not in _PROG_CACHE:
        _PROG_CACHE[key] = _build_program()
    return _PROG_CACHE[key]


def _mats_np():
    """Stationary matrices, packed [128, 386] bf16.

    matmul convention: out[q, f] = sum_p lhsT[p, q] * rhs[p, f].
    """
    import ml_dtypes

    m = np.zeros((128, 5 * 128 + 2), dtype=np.float32)
    Bz = m[:, 0:128]
    A1 = m[:, 128:256]
    A2 = m[:, 256:384]
    PL = m[:, 384:386]
    for dd in range(63):
        for s in range(2):
            q = 2 * dd + s
            Bz[2 * dd + 2 + s, q] = 1.0
            Bz[2 * dd + s, q] = -1.0
    Bz[0:64, 126] = 1.0  # crown indicator (d < 32  <->  p < 64)
    Bz[:, 127] = 1.0     # ones (total)
    for dd in range(64):
        A1[2 * dd + 1, 2 * dd] = 1.0
        A2[2 * dd, 2 * dd] = -1.0
    PL[0, 0] = PL[1, 0] = 1.0      # plane d=0  (p in {0,1})
    PL[126, 1] = PL[127, 1] = 1.0  # plane d=63 (p in {126,127})
    np.fill_diagonal(m[:, 386:514], 1.0)   # I
    np.fill_diagonal(m[:, 514:642], -1.0)  # -I
    return m.astype(ml_dtypes.bfloat16)


def _combine(partials, b=B, c=C, d=D, h=H, w=W):
    """Host-side finish: per-core [128, 144] fp32 partials -> [3] fp32."""
    nslice = b * c
    jpc = nslice // len(partials)

    crown = np.zeros(nslice, dtype=np.float64)
    root = np.zeros(nslice, dtype=np.float64)
    gx_sum = 0.0
    gy_sum = 0.0
    gz_sum = 0.0
    for k, p in enumerate(partials):
        p = p.astype(np.float64)
        gz_sum -= p[1, PLC] - p[0, PLC]  # global sum(dz) telescope
        for j in range(jpc):
            g = slice(GZ0 + j * NT, GZ0 + (j + 1) * NT)
            crown_j = p[126, g].sum()
            total_j = p[127, g].sum()
            gzrelu_j = p[0:126, g].sum()
            gz_sum += 2.0 * gzrelu_j

            g0_j = p[:, GYS0 + j].sum()
            g1_j = p[:, GYS1 + j].sum()
            if j in GY_DMAX:
                # max formulation: sum|a-b| = 2*sum(max) - sum(a) - sum(b)
                gymax_j = p[:, GYM + j].sum()
                gy_int = 2.0 * gymax_j - (total_j - g0_j) - (total_j - g1_j)
            else:
                gyrelu_j = p[:, GYR + j].sum()
                gy_int = 2.0 * gyrelu_j - (g1_j - g0_j)
            pbrelu_j = p[:, PBR + j].sum()
            pbsum_j = p[1::2, GYS0 + j].sum() - p[0::2, GYS1 + j].sum()
            gy_sum += gy_int + (2.0 * pbrelu_j - pbsum_j)

            c0_j = p[:, GXC0 + j].sum()
            c1_j = p[:, GXC1 + j].sum()
            if j in GXPE_SLICES:
                jx = j - GXPE_SLICES[0]
                gp = slice(GXP + jx * NT, GXP + (jx + 1) * NT)
                relu_true = p[:, gp].sum() - p[:, GXB + j].sum()
                # sum(dx_true) telescopes to colsum(w-1) - colsum(0)
                gx_sum += 2.0 * relu_true - (c1_j - c0_j)
            else:
                # STT route sums true-pair maxes; DMAX route sums flat
                # maxes including bogus w-wrap pairs (GXB col corrects)
                gxmax_j = p[:, GX + j].sum() - p[:, GXB + j].sum()
                gx_sum += 2.0 * gxmax_j - (total_j - c0_j) - (total_j - c1_j)

            crown[k * jpc + j] = crown_j
            root[k * jpc + j] = total_j - crown_j

    total = crown + root
    valid = (total > 0) & (root > 0)
    safe_root = np.where(root > 0, root, 1.0)
    ratio_loss = np.where(valid, (crown / safe_root - EXPECTED_RATIO) ** 2, 0.0)
    cr_loss = ratio_loss.sum() / nslice

    nx = nslice * d * h * (w - 1)
    ny = nslice * d * (h - 1) * w
    nz = nslice * (d - 1) * h * w
    tv = gx_sum / nx + gy_sum / ny + gz_sum / nz

    crown_root = cr_loss * CROWN_ROOT_W
    smoothness = tv * SMOOTH_W
    return np.array(
        [crown_root, smoothness, crown_root + smoothness], dtype=np.float32
    )


def kernel(segmentation: np.ndarray) -> np.ndarray:
    global last_exec_time_ns
    from concourse.bass_utils import run_bass_kernel_spmd

    seg = np.ascontiguousarray(np.asarray(segmentation), dtype=np.float32)
    assert seg.shape == (B, C, D, H, W)
    nc = _get_program()

    mats = _mats_np()
    shards = seg.reshape(B * C, D, H, W)
    in_maps = [
        {"seg": np.ascontiguousarray(shards[k * JPC : (k + 1) * JPC]), "mats": mats}
        for k in range(NCORES)
    ]
    trace = bool(os.environ.get("BASS_TRACE"))
    res = run_bass_kernel_spmd(nc, in_maps, list(range(NCORES)), trace=trace)
    last_exec_time_ns = res.exec_time_ns
    partials = [res.results[k]["partials"] for k in range(NCORES)]
    return _combine(partials)
